# revision 10
# baseline (speedup 1.0000x reference)
"""Trainium2 Bass kernel for ConsistencyMaskFromBoxes.

Computes 0.1 * mean(BCEWithLogits(seg_preds, union_of_boxes_mask)) across
B=32 images of 640x640, data-parallel over 8 NeuronCores (4 images/core).

Math: for y in {0,1}, max(x,0) - x*y + log1p(exp(-|x|)) == softplus(x) - x*y.
So per core we need Sum(softplus(x)) - Sum(x * y), where
y[h,w] = min(1, count[h,w]) and count = iny^T @ inx summed over the image's
boxes (iny/inx are 0/1 row/col interval indicators per box). count is a tiny
PE matmul; min+mul+row-reduce is one fused DVE op; softplus+row-reduce is one
fused ACT op. The only heavy memory traffic is reading seg_preds once.
"""

import os
import sys

sys.path.insert(0, "/opt/trn_rl_repo")

import numpy as np

B, H, W = 32, 640, 640
N_CORES = 8
IMGS = B // N_CORES          # images per core
RB = H // 128                # 128-row blocks per image
WEIGHT = 0.1

_nc_cache = {}

LAST_RESULTS = None  # test harness introspection


def _layout(s_raw):
    """Uniform (SPMD-identical) slot layout for box indicator masks.

    Returns (n_tiles, chunks) where chunks[img] is a list of
    (tile_idx, partition_offset, size) k-chunks for that image's matmul.
    """
    if s_raw <= 128:
        s = 32 if s_raw <= 32 else (64 if s_raw <= 64 else 128)
        per_tile = 128 // s
        n_tiles = (IMGS + per_tile - 1) // per_tile
        chunks = [[(img // per_tile, (img % per_tile) * s, s)] for img in range(IMGS)]
    else:
        n_chunks = (s_raw + 127) // 128
        n_tiles = IMGS * n_chunks
        chunks = [
            [(img * n_chunks + ci, 0, 128) for ci in range(n_chunks)]
            for img in range(IMGS)
        ]
    return n_tiles, chunks


def _build(s_raw):
    import concourse.bacc as bacc
    import concourse.tile as tile
    from concourse import mybir

    n_tiles, chunks = _layout(s_raw)

    f32 = mybir.dt.float32
    bf16 = mybir.dt.bfloat16
    i32 = mybir.dt.int32
    A = mybir.AluOpType
    AF = mybir.ActivationFunctionType
    AX = mybir.AxisListType

    nc = bacc.Bacc("TRN2", target_bir_lowering=False, debug=False,
                   enable_asserts=False, num_devices=N_CORES)

    x_d = nc.dram_tensor("x", [IMGS * H, W], f32, kind="ExternalInput")
    c_d = nc.dram_tensor("coords", [n_tiles, 128, 4], f32, kind="ExternalInput")
    o_d = nc.dram_tensor("out", [1, 2], f32, kind="ExternalOutput")

    with tile.TileContext(nc) as tc:
        with (
            tc.tile_pool(name="xp", bufs=3) as xp,
            tc.tile_pool(name="sp", bufs=1) as sp,
            tc.tile_pool(name="mp", bufs=1) as mp,
            tc.tile_pool(name="accp", bufs=1) as accp,
            tc.tile_pool(name="pp", bufs=3, space="PSUM") as pp,
            tc.tile_pool(name="pfin", bufs=1, space="PSUM") as pfin,
        ):
            # --- box indicator masks, built on the (otherwise idle) gpsimd ---
            iota_i = mp.tile([128, W], i32, tag="iota_i")
            nc.gpsimd.iota(iota_i[:], pattern=[[1, W]], base=0, channel_multiplier=0)
            iota_f = mp.tile([128, W], f32, tag="iota_f")
            nc.gpsimd.tensor_copy(iota_f[:], iota_i[:])

            inys, inxs = [], []
            for t in range(n_tiles):
                co = mp.tile([128, 4], f32, tag=f"co{t}")
                nc.gpsimd.dma_start(co[:], c_d.ap()[t])
                iny = mp.tile([128, W], bf16, tag=f"iny{t}")
                inx = mp.tile([128, W], bf16, tag=f"inx{t}")
                tmp = sp.tile([128, W], bf16, tag="tmp")
                tmp2 = sp.tile([128, W], bf16, tag="tmp2")
                nc.gpsimd.tensor_scalar(tmp[:], iota_f[:], co[:, 0:1], None, A.is_ge)
                nc.gpsimd.tensor_scalar(tmp2[:], iota_f[:], co[:, 1:2], None, A.is_le)
                nc.gpsimd.tensor_tensor(iny[:], tmp[:], tmp2[:], A.mult)
                nc.gpsimd.tensor_scalar(tmp[:], iota_f[:], co[:, 2:3], None, A.is_ge)
                nc.gpsimd.tensor_scalar(tmp2[:], iota_f[:], co[:, 3:4], None, A.is_le)
                nc.gpsimd.tensor_tensor(inx[:], tmp[:], tmp2[:], A.mult)
                inys.append(iny)
                inxs.append(inx)

            ones = mp.tile([128, 1], f32, tag="ones")
            nc.gpsimd.memset(ones[:], 1.0)

            sacc = accp.tile([128, IMGS], f32, tag="sacc")
            zacc = accp.tile([128, IMGS * RB], f32, tag="zacc")
            u_out = sp.tile([128, RB * W], f32, tag="u_out")
            l_out = sp.tile([128, RB * W], f32, tag="l_out")

            for img in range(IMGS):
                xi = xp.tile([128, RB * W], f32, tag="x")
                src = x_d.ap()[img * H:(img + 1) * H, :].rearrange(
                    "(a p) w -> p a w", p=128)
                nc.sync.dma_start(xi[:].rearrange("p (a w) -> p a w", a=RB), src)

                # softplus(x) = ln(1 + exp(x)); exp/ln share one ACT table set.
                # accum_out gives per-partition row sums of softplus for free.
                nc.scalar.activation(u_out[:], xi[:], AF.Exp)
                nc.scalar.activation(l_out[:], u_out[:], AF.Ln, bias=1.0,
                                     accum_out=sacc[:, img:img + 1])

                ch = chunks[img]
                for rb in range(RB):
                    cnt = pp.tile([128, W], f32, tag="cnt")
                    for (w0, w1) in ((0, 512), (512, W)):
                        for ci, (t, off, size) in enumerate(ch):
                            nc.tensor.matmul(
                                cnt[:, w0:w1],
                                inys[t][off:off + size, rb * 128:(rb + 1) * 128],
                                inxs[t][off:off + size, w0:w1],
                                start=(ci == 0), stop=(ci == len(ch) - 1),
                                tile_position=(off, 0))
                    # z = min(cnt,1) * x ; zacc col = row-sum(z)
                    zo = sp.tile([128, W], f32, tag="zo")
                    col = img * RB + rb
                    nc.vector.scalar_tensor_tensor(
                        zo[:], cnt[:], 1.0, xi[:, rb * W:(rb + 1) * W],
                        A.min, A.mult, accum_out=zacc[:, col:col + 1])

            # column 0 = row sums of softplus, column 1 = row sums of x*y;
            # a [1,2] matmul against ones finishes the partition reduction.
            r2 = accp.tile([128, 2], f32, tag="r2")
            nc.vector.tensor_reduce(r2[:, 0:1], sacc[:], AX.X, A.add)
            nc.vector.tensor_reduce(r2[:, 1:2], zacc[:], AX.X, A.add)
            fin = pfin.tile([1, 2], f32, tag="fin")
            nc.tensor.matmul(fin[:], ones[:], r2[:], start=True, stop=True)
            fin_sb = accp.tile([1, 2], f32, tag="fin_sb")
            nc.vector.tensor_copy(fin_sb[:], fin[:])
            nc.sync.dma_start(o_d.ap()[:, :], fin_sb[:])

    nc.compile()
    return nc


def _prepare(seg_preds, bboxes, batch_idx, is_seg):
    x = np.ascontiguousarray(np.asarray(seg_preds, dtype=np.float32)).reshape(B, H, W)
    bb = np.asarray(bboxes, dtype=np.float32)
    bidx = np.asarray(batch_idx).astype(np.int64)
    seg = np.asarray(is_seg).astype(bool)

    # box coords exactly as the reference computes them (f32 ops, trunc to int)
    cx = bb[:, 0] * np.float32(W)
    cy = bb[:, 1] * np.float32(H)
    bw = bb[:, 2] * np.float32(W)
    bh = bb[:, 3] * np.float32(H)
    two = np.float32(2.0)
    x1 = np.clip(cx - bw / two, 0, W - 1).astype(np.int32)
    y1 = np.clip(cy - bh / two, 0, H - 1).astype(np.int32)
    x2 = np.clip(cx + bw / two, 0, W - 1).astype(np.int32)
    y2 = np.clip(cy + bh / two, 0, H - 1).astype(np.int32)

    active = ~seg[bidx]
    counts = np.bincount(bidx[active], minlength=B)
    s_raw = int(max(1, counts.max()))
    n_tiles, chunks = _layout(s_raw)

    # sentinel (y1=1,y2=0) rasterizes to an empty row/col indicator
    coords = np.zeros((N_CORES, n_tiles, 128, 4), dtype=np.float32)
    coords[..., 0] = 1.0
    coords[..., 2] = 1.0

    fill = np.zeros(B, dtype=np.int64)  # next free slot index per image
    for m in np.nonzero(active)[0]:
        b = int(bidx[m])
        core, li = b // IMGS, b % IMGS
        j = int(fill[b])
        fill[b] += 1
        ci = j // 128 if s_raw > 128 else 0
        t, off, size = chunks[li][ci]
        pos = off + (j - ci * 128 if s_raw > 128 else j)
        coords[core, t, pos] = (y1[m], y2[m], x1[m], x2[m])

    in_maps = [
        {"x": np.ascontiguousarray(x[c * IMGS:(c + 1) * IMGS].reshape(IMGS * H, W)),
         "coords": np.ascontiguousarray(coords[c])}
        for c in range(N_CORES)
    ]
    has_det = 1.0 if bool(np.any(~seg)) else 0.0
    return in_maps, has_det, s_raw


def kernel(seg_preds, bboxes, batch_idx, is_seg):
    from concourse.bass_utils import run_bass_kernel_spmd

    in_maps, has_det, s_raw = _prepare(seg_preds, bboxes, batch_idx, is_seg)
    key = _layout(s_raw)[0]
    if key not in _nc_cache:
        _nc_cache[key] = _build(s_raw)
    nc = _nc_cache[key]

    trace = bool(os.environ.get("KERNEL_TRACE"))
    res = run_bass_kernel_spmd(nc, in_maps, list(range(N_CORES)), trace=trace)
    global LAST_RESULTS
    LAST_RESULTS = res

    total = float(np.sum(np.array(
        [np.float64(res.results[c]["out"][0, 0]) -
         np.float64(res.results[c]["out"][0, 1]) for c in range(N_CORES)])))
    val = np.float32(np.float32(WEIGHT) * np.float32(total / (B * H * W)) *
                     np.float32(has_det))
    return np.asarray(val, dtype=np.float32)


# revision 11
# speedup vs baseline: 1.8574x; 1.8574x over previous
"""Trainium2 Bass kernel for ConsistencyMaskFromBoxes.

Computes 0.1 * mean(BCEWithLogits(seg_preds, union_of_boxes_mask)) across
B=32 images of 640x640, data-parallel over 8 NeuronCores (4 images/core).

Math: for y in {0,1}, max(x,0) - x*y + log1p(exp(-|x|)) == softplus(x) - x*y.
So per core we need Sum(softplus(x)) - Sum(x * y), where
y[h,w] = min(1, count[h,w]) and count = iny^T @ inx summed over the image's
boxes (iny/inx are 0/1 row/col interval indicators per box, shipped as tiny
bf16 inputs). count is a small PE matmul per 128-row block; min+mul+row-reduce
is one fused DVE op; softplus = ln(1+exp(x)) is two ACT ops (exp & ln share
one activation table) with the row-reduction fused into the ln's accumulator.
The only heavy memory traffic is reading seg_preds once.
"""

import os
import sys

sys.path.insert(0, "/opt/trn_rl_repo")

import numpy as np

B, H, W = 32, 640, 640
N_CORES = 8
IMGS = B // N_CORES          # images per core
RB = H // 128                # 128-row blocks per image
WEIGHT = 0.1

# (img, rb_start, n_rb) — image 0 is split so the first ACT op can start
# as soon as the first 0.65 MB of DMA lands instead of 1.6 MB.
SEGMENTS = [(0, 0, 2), (0, 2, 3)] + [(i, 0, RB) for i in range(1, IMGS)]

_nc_cache = {}

LAST_RESULTS = None  # test harness introspection


def _layout(s_raw):
    """Uniform (SPMD-identical) slot layout for box indicator masks.

    Returns (n_tiles, chunks) where chunks[img] is a list of
    (tile_idx, partition_offset, size) k-chunks for that image's matmul.
    """
    if s_raw <= 128:
        s = 32 if s_raw <= 32 else (64 if s_raw <= 64 else 128)
        per_tile = 128 // s
        n_tiles = (IMGS + per_tile - 1) // per_tile
        chunks = [[(img // per_tile, (img % per_tile) * s, s)] for img in range(IMGS)]
    else:
        n_chunks = (s_raw + 127) // 128
        n_tiles = IMGS * n_chunks
        chunks = [
            [(img * n_chunks + ci, 0, 128) for ci in range(n_chunks)]
            for img in range(IMGS)
        ]
    return n_tiles, chunks


def _build(s_raw):
    import concourse.bacc as bacc
    import concourse.tile as tile
    from concourse import mybir

    n_tiles, chunks = _layout(s_raw)

    f32 = mybir.dt.float32
    bf16 = mybir.dt.bfloat16
    A = mybir.AluOpType
    AF = mybir.ActivationFunctionType
    AX = mybir.AxisListType

    nc = bacc.Bacc("TRN2", target_bir_lowering=False, debug=False,
                   enable_asserts=False, num_devices=N_CORES)

    x_d = nc.dram_tensor("x", [IMGS * H, W], f32, kind="ExternalInput")
    m_d = nc.dram_tensor("masks", [2, n_tiles, 128, W], bf16,
                         kind="ExternalInput")
    o_d = nc.dram_tensor("out", [1, 2], f32, kind="ExternalOutput")

    with tile.TileContext(nc) as tc:
        with (
            tc.tile_pool(name="xp", bufs=1) as xp,
            tc.tile_pool(name="sp", bufs=1) as sp,
            tc.tile_pool(name="mp", bufs=1) as mp,
            tc.tile_pool(name="accp", bufs=1) as accp,
            tc.tile_pool(name="pp", bufs=3, space="PSUM") as pp,
            tc.tile_pool(name="pfin", bufs=1, space="PSUM") as pfin,
        ):
            # host-built 0/1 interval indicators arrive over the gpsimd ring,
            # in parallel with the seg_preds DMAs on the sync ring
            inys, inxs = [], []
            for t in range(n_tiles):
                iny = mp.tile([128, W], bf16, tag=f"iny{t}")
                inx = mp.tile([128, W], bf16, tag=f"inx{t}")
                nc.gpsimd.dma_start(iny[:], m_d.ap()[0, t])
                nc.gpsimd.dma_start(inx[:], m_d.ap()[1, t])
                inys.append(iny)
                inxs.append(inx)

            ones = mp.tile([128, 1], f32, tag="ones")
            nc.gpsimd.memset(ones[:], 1.0)

            sacc = accp.tile([128, len(SEGMENTS)], f32, tag="sacc")
            zacc = accp.tile([128, IMGS * RB], f32, tag="zacc")

            # prefetch every segment up-front; unique tags = no slot reuse,
            # so DMA issue is never gated on compute
            xs = []
            for si, (img, rb0, nrb) in enumerate(SEGMENTS):
                xi = xp.tile([128, nrb * W], f32, tag=f"xs{si}")
                src = x_d.ap()[img * H:(img + 1) * H, :].rearrange(
                    "(a p) w -> p a w", p=128)
                nc.sync.dma_start(
                    xi[:].rearrange("p (a w) -> p a w", a=nrb),
                    src[:, rb0:rb0 + nrb, :])
                xs.append(xi)

            for si, (img, rb0, nrb) in enumerate(SEGMENTS):
                xi = xs[si]
                # softplus(x) = ln(1 + exp(x)); row sums via ln's accumulator
                u = sp.tile([128, nrb * W], f32, tag="u")
                l = sp.tile([128, nrb * W], f32, tag="l")
                nc.scalar.activation(u[:], xi[:], AF.Exp)
                nc.scalar.activation(l[:], u[:], AF.Ln, bias=1.0,
                                     accum_out=sacc[:, si:si + 1])

                ch = chunks[img]
                for r in range(nrb):
                    rb = rb0 + r
                    cnt = pp.tile([128, W], f32, tag="cnt")
                    for (w0, w1) in ((0, 512), (512, W)):
                        for ci, (t, off, size) in enumerate(ch):
                            nc.tensor.matmul(
                                cnt[:, w0:w1],
                                inys[t][off:off + size, rb * 128:(rb + 1) * 128],
                                inxs[t][off:off + size, w0:w1],
                                start=(ci == 0), stop=(ci == len(ch) - 1),
                                tile_position=(off, 0))
                    # z = min(cnt,1) * x ; zacc col = row-sum(z)
                    zo = sp.tile([128, W], f32, tag="zo")
                    col = img * RB + rb
                    nc.vector.scalar_tensor_tensor(
                        zo[:], cnt[:], 1.0, xi[:, r * W:(r + 1) * W],
                        A.min, A.mult, accum_out=zacc[:, col:col + 1])

            # column 0 = total softplus, column 1 = total x*y;
            # a [1,2] matmul against ones finishes the partition reduction.
            r2 = accp.tile([128, 2], f32, tag="r2")
            nc.vector.tensor_reduce(r2[:, 0:1], sacc[:], AX.X, A.add)
            nc.vector.tensor_reduce(r2[:, 1:2], zacc[:], AX.X, A.add)
            fin = pfin.tile([1, 2], f32, tag="fin")
            nc.tensor.matmul(fin[:], ones[:], r2[:], start=True, stop=True)
            fin_sb = accp.tile([1, 2], f32, tag="fin_sb")
            nc.vector.tensor_copy(fin_sb[:], fin[:])
            nc.sync.dma_start(o_d.ap()[:, :], fin_sb[:])

    # Exp and Ln both live in the natural_log_exp_and_others activation
    # table, but the default chooser resolves each function to the first
    # table containing it, reloading tables between every exp and ln.
    # Strip Exp/Ln from all other tables (positions preserved) so exactly
    # one ACT_TABLE_LOAD is emitted.
    orig_gat = bacc.get_activation_tables

    def pinned(arch):
        out = {}
        for name, funcs in orig_gat(arch).items():
            if name == "natural_log_exp_and_others":
                out[name] = funcs
            else:
                out[name] = {f for f in funcs if f.name not in ("Exp", "Ln")}
        return out

    bacc.get_activation_tables = pinned
    try:
        nc.compile()
    finally:
        bacc.get_activation_tables = orig_gat
    return nc


def _prepare(seg_preds, bboxes, batch_idx, is_seg):
    import ml_dtypes

    x = np.ascontiguousarray(np.asarray(seg_preds, dtype=np.float32)).reshape(B, H, W)
    bb = np.asarray(bboxes, dtype=np.float32)
    bidx = np.asarray(batch_idx).astype(np.int64)
    seg = np.asarray(is_seg).astype(bool)

    # box coords exactly as the reference computes them (f32 ops, trunc to int)
    cx = bb[:, 0] * np.float32(W)
    cy = bb[:, 1] * np.float32(H)
    bw = bb[:, 2] * np.float32(W)
    bh = bb[:, 3] * np.float32(H)
    two = np.float32(2.0)
    x1 = np.clip(cx - bw / two, 0, W - 1).astype(np.int32)
    y1 = np.clip(cy - bh / two, 0, H - 1).astype(np.int32)
    x2 = np.clip(cx + bw / two, 0, W - 1).astype(np.int32)
    y2 = np.clip(cy + bh / two, 0, H - 1).astype(np.int32)

    active = ~seg[bidx]
    counts = np.bincount(bidx[active], minlength=B)
    s_raw = int(max(1, counts.max()))
    n_tiles, chunks = _layout(s_raw)

    # 0/1 row/col interval indicators per box slot; unused slots stay zero
    masks = np.zeros((N_CORES, 2, n_tiles, 128, W), dtype=np.float32)
    hh = np.arange(H)
    ww = np.arange(W)
    fill = np.zeros(B, dtype=np.int64)  # next free slot index per image
    for m in np.nonzero(active)[0]:
        b = int(bidx[m])
        core, li = b // IMGS, b % IMGS
        j = int(fill[b])
        fill[b] += 1
        ci = j // 128 if s_raw > 128 else 0
        t, off, size = chunks[li][ci]
        pos = off + (j - ci * 128 if s_raw > 128 else j)
        masks[core, 0, t, pos] = (hh >= y1[m]) & (hh <= y2[m])
        masks[core, 1, t, pos] = (ww >= x1[m]) & (ww <= x2[m])
    masks_bf = masks.astype(ml_dtypes.bfloat16)

    in_maps = [
        {"x": np.ascontiguousarray(x[c * IMGS:(c + 1) * IMGS].reshape(IMGS * H, W)),
         "masks": np.ascontiguousarray(masks_bf[c])}
        for c in range(N_CORES)
    ]
    has_det = 1.0 if bool(np.any(~seg)) else 0.0
    return in_maps, has_det, s_raw


def kernel(seg_preds, bboxes, batch_idx, is_seg):
    from concourse.bass_utils import run_bass_kernel_spmd

    in_maps, has_det, s_raw = _prepare(seg_preds, bboxes, batch_idx, is_seg)
    key = _layout(s_raw)[0]
    if key not in _nc_cache:
        _nc_cache[key] = _build(s_raw)
    nc = _nc_cache[key]

    trace = bool(os.environ.get("KERNEL_TRACE"))
    res = run_bass_kernel_spmd(nc, in_maps, list(range(N_CORES)), trace=trace)
    global LAST_RESULTS
    LAST_RESULTS = res

    total = float(np.sum(np.array(
        [np.float64(res.results[c]["out"][0, 0]) -
         np.float64(res.results[c]["out"][0, 1]) for c in range(N_CORES)])))
    val = np.float32(np.float32(WEIGHT) * np.float32(total / (B * H * W)) *
                     np.float32(has_det))
    return np.asarray(val, dtype=np.float32)
